# revision 17
# baseline (speedup 1.0000x reference)
"""StyleGAN-style modulated 3x3 conv on 8 Trainium2 NeuronCores.

Problem: y = conv2d(x, kernel * (style+1) / demod), SAME padding,
  x [B=8, H=128, W=128, C=256], kernel [3,3,C=256,F=256],
  style [B,1,1,C], demod[f] = sqrt(sum_{ky,kx,c} wmod^2 + 1e-8).

Sharding: data-parallel over batch B — each of the 8 cores convolves one
sample with its own modulated/demodulated kernel. No cross-core comm.

Algorithm (per core): 1D Winograd F(2,3) along H + direct 3-tap conv
along W, all matmuls in bf16 (rel-err gate is 2e-2; this lands ~6e-3).
PE work is 2/3 of the direct implicit-GEMM floor.

  - input transform (DVE, bf16 2x): T0=d0-d2, T1=d1+d2, T2=d2-d1,
    T3=d1-d3 where d_k = x_pad[c, 2*ht+k, :]. x streamed in 10-row
    slots (8 fresh + 2 halo).
  - weight transform (DVE, bf16, split per f-half): W0=m0,
    W1=(m0+m1+m2)/2, W2=(m0-m1+m2)/2, W3=m2, m_ky = raw[ky]*(style+1).
  - GEMM (PE): M_j[f,p] += sum_{kx,c} T_j[c, p+kx-1]*W_j[kx,c,f], PSUM
    [f=128, npx<=512]. Schedule is f-half-phased (all fh0 groups, then
    fh1) so only 3 of 12 weight-DMA chunks gate the conv start; first
    group is 256 px (gated by a 6-row x mini-DMA); last groups shrink
    (384/320/192) so the final stores are small.
  - output transform + demod fused in the drain: ACT copies each M_j
    to SBUF bf16 scaled by invd[f] (per-partition scalar), DVE combines
    at bf16 2x: y_e=(c0+c1)+c2, y_o=(c1-c2)-c3. Stores rotate over the
    3 DMA queues (gpsimd/sync/scalar).
  - demod: sq=(raw*s)^2 per f-half (ACT), ones-column matmuls (18 per
    f-half) emitted right after that phase's first group.
  - 8 dummy matmuls at kernel start warm the PE HAM clock gate to
    2.4 GHz before the first real matmul issues.

Emission order is tuned to Tile's positional counting semaphores: each
engine queue executes in emission order, so every DVE/ACT op is emitted
in exactly the order the pipeline should consume it.

Host does layout-only marshalling (shard over B, transpose+pad+cast);
un-interleaves parity planes + strips pad columns on gather.
"""

import sys
import os

for _p in ("/opt/trn_rl_repo", "/root/.axon_site", "/root/.axon_site/_ro/trn_rl_repo",
           "/root/.axon_site/_ro/pypackages"):
    if os.path.isdir(_p) and _p not in sys.path:
        sys.path.append(_p)

import numpy as np
import ml_dtypes

B, H, W, C, F = 8, 128, 128, 256, 256
CH = C // 128                  # contraction halves
FHN = F // 128                 # f halves
NTAP = 9
WW = W + 2                     # padded width  (w = -1..128)
HP = H + 2                     # padded height (h = -1..128)
NHT = H // 2                   # 64 h-tiles (output row pairs)
TLEN = NHT * WW                # 8320 flat transformed positions
GT = 8                         # zero guard around T planes (+-1 shifts)
NSLOT = 16                     # x slots: 8 fresh rows + 2 halo rows each
SROWS = 10
CHT = 4                        # h-tiles transformed per chunk (= 1 slot)
CLEN = CHT * WW                # 520
# group sizes per f-half: small first group (fast start), small tail
GRP = [256] + [512] * 14 + [384, 320, 192]
assert sum(GRP) == TLEN
N_CORES = 8

_COMPILED = {}


def _build_nc():
    import concourse.bacc as bacc
    import concourse.mybir as mybir
    import concourse.tile as tile

    f32 = mybir.dt.float32
    bf16 = mybir.dt.bfloat16
    AF = mybir.ActivationFunctionType

    nc = bacc.Bacc("TRN2", target_bir_lowering=False, debug=False,
                   num_devices=N_CORES)

    xt_d = nc.dram_tensor("xt", [CH, 128, HP * WW], bf16,
                          kind="ExternalInput").ap()
    st_d = nc.dram_tensor("st", [128, CH], f32, kind="ExternalInput").ap()
    wk_d = nc.dram_tensor("wk", [CH, 128, NTAP, F], bf16,
                          kind="ExternalInput").ap()
    # yt[f_half][f][parity][flat ht*WW+w+1]; pad cols stripped on host
    yt_d = nc.dram_tensor("yt", [FHN, 128, 2, TLEN], bf16,
                          kind="ExternalOutput").ap()

    with tile.TileContext(nc) as tc:
        with tc.tile_pool(name="pers", bufs=1) as pers, \
             tc.tile_pool(name="xs", bufs=3) as xs, \
             tc.tile_pool(name="wtmp", bufs=1) as wtmp, \
             tc.tile_pool(name="dtmp", bufs=2) as dtmp, \
             tc.tile_pool(name="stage", bufs=3) as stage, \
             tc.tile_pool(name="ps", bufs=7, space="PSUM") as ps, \
             tc.tile_pool(name="psd", bufs=1, space="PSUM") as psd:

            # ---- HAM warm-up: PE busy from the preamble end so the
            # clock gate is at 2.4 GHz when real matmuls arrive ----
            dm_t = pers.tile([128, 512], bf16, tag="dm", name="dm_t")
            nc.gpsimd.memset(dm_t[:], 0.0)
            warm = psd.tile([128, 512], f32, tag="d2", name="warm")
            for _ in range(8):
                nc.tensor.matmul(warm[:], dm_t[:, :128], dm_t[:],
                                 start=True, stop=True)

            # ---- style scalars ----
            s_t = pers.tile([128, CH], f32, tag="s", name="s_t")
            nc.sync.dma_start(s_t[:], st_d)
            nc.vector.tensor_scalar_add(s_t[:], s_t[:], 1.0)
            hs_t = pers.tile([128, CH], f32, tag="hs", name="hs_t")
            nc.vector.tensor_scalar_mul(hs_t[:], s_t[:], 0.5)

            # ---- T planes; guards zeroed on gpsimd (keeps DVE queue
            # short — DVE position gates the first matmul) ----
            tp = [[pers.tile([128, GT + TLEN + GT], bf16, tag=f"T{j}_{ch}",
                             name=f"T{j}_{ch}") for ch in range(CH)]
                  for j in range(4)]
            for j in range(4):
                for ch in range(CH):
                    nc.gpsimd.memset(tp[j][ch][:, 0:GT], 0.0)
                    nc.gpsimd.memset(tp[j][ch][:, GT + TLEN:], 0.0)

            # ---- weight DMA (sync), split per (ky, ch, f-half):
            # fh0 chunks first, ordered ky0, ky2, ky1 ----
            wraw = [pers.tile([128, NTAP, F], bf16, tag=f"wraw{ch}",
                              name=f"wraw{ch}") for ch in range(CH)]

            def dma_wk(ky, fh):
                for ch in range(CH):
                    nc.sync.dma_start(
                        wraw[ch][:, 3 * ky:3 * ky + 3,
                                 fh * 128:(fh + 1) * 128],
                        wk_d[ch][:, 3 * ky:3 * ky + 3,
                                 fh * 128:(fh + 1) * 128])

            # ---- x slot DMA + input transform helpers ----
            slot_tiles = {}

            def emit_slot_dma(s):
                if s >= NSLOT or s in slot_tiles:
                    return
                tl = []
                for ch in range(CH):
                    t = xs.tile([128, SROWS, WW], bf16, tag=f"x{ch}",
                                name=f"x{s}_{ch}")
                    eng = nc.scalar if ch == 0 else nc.gpsimd
                    base = 8 * s * WW
                    if s == 0:
                        # split: 6-row mini-DMA unblocks the first
                        # transform sub-chunk (first conv group)
                        eng.dma_start(t[:, 0:6], xt_d[ch][:, 0:6 * WW])
                        eng.dma_start(t[:, 6:SROWS],
                                      xt_d[ch][:, 6 * WW:SROWS * WW])
                    else:
                        eng.dma_start(
                            t[:], xt_d[ch][:, base:base + SROWS * WW])
                    tl.append(t)
                slot_tiles[s] = tl

            done_chunks = set()

            def transform_piece(c, ch, r0, ht0, nht):
                """Transform nht h-tiles of chunk c, channel ch, reading
                slot rows starting at local row r0."""
                sl = slot_tiles[c][ch]
                o = [tp[j][ch][:, GT + CLEN * c + ht0 * WW:
                               GT + CLEN * c + (ht0 + nht) * WW]
                     .rearrange("p (a b) -> p a b", a=nht)
                     for j in range(4)]
                d = [sl[:, r0 + k:r0 + k + 2 * nht - 1:2, :]
                     for k in range(4)]
                nc.vector.tensor_sub(o[0], d[0], d[2])
                nc.vector.tensor_add(o[1], d[1], d[2])
                nc.vector.tensor_sub(o[2], d[2], d[1])
                nc.vector.tensor_sub(o[3], d[1], d[3])

            def emit_transform(c, ch=None):
                if c >= NSLOT or c in done_chunks:
                    return
                if ch is None:
                    done_chunks.add(c)
                    emit_slot_dma(c + 3)
                    for cc in range(CH):
                        transform_piece(c, cc, 0, 0, CHT)
                    return
                transform_piece(c, ch, 0, 0, CHT)

            # ---- weight transform (split per f-half) ----
            wt = [[pers.tile([128, 3, F], bf16, tag=f"wt{j}_{ch}",
                             name=f"wt{j}_{ch}") for ch in range(CH)]
                  for j in range(4)]

            def combos_j0(fh):
                fsl = slice(fh * 128, (fh + 1) * 128)
                for ch in range(CH):
                    nc.vector.tensor_scalar_mul(
                        wt[0][ch][:, :, fsl], wraw[ch][:, 0:3, fsl],
                        s_t[:, ch:ch + 1])

            def combos_j3(fh):
                fsl = slice(fh * 128, (fh + 1) * 128)
                for ch in range(CH):
                    nc.vector.tensor_scalar_mul(
                        wt[3][ch][:, :, fsl], wraw[ch][:, 6:9, fsl],
                        s_t[:, ch:ch + 1])

            def combos_j12(fh):
                fsl = slice(fh * 128, (fh + 1) * 128)
                for ch in range(CH):
                    su = wtmp.tile([128, 3, 128], bf16, tag="su", name="su")
                    nc.vector.tensor_add(su[:], wraw[ch][:, 0:3, fsl],
                                         wraw[ch][:, 6:9, fsl])
                    sv = wtmp.tile([128, 3, 128], bf16, tag="sv", name="sv")
                    nc.vector.tensor_add(sv[:], su[:], wraw[ch][:, 3:6, fsl])
                    nc.vector.tensor_scalar_mul(wt[1][ch][:, :, fsl], sv[:],
                                                hs_t[:, ch:ch + 1])
                    sw = wtmp.tile([128, 3, 128], bf16, tag="sw", name="sw")
                    nc.vector.tensor_sub(sw[:], su[:], wraw[ch][:, 3:6, fsl])
                    nc.vector.tensor_scalar_mul(wt[2][ch][:, :, fsl], sw[:],
                                                hs_t[:, ch:ch + 1])

            # ---- demod state ----
            sq = [pers.tile([128, NTAP, F], bf16, tag=f"sq{ch}",
                            name=f"sq{ch}") for ch in range(CH)]
            ones_t = pers.tile([128, 1], bf16, tag="ones", name="ones_t")
            nc.gpsimd.memset(ones_t[:], 1.0)
            eps_t = pers.tile([128, 1], f32, tag="eps", name="eps_t")
            nc.gpsimd.memset(eps_t[:], 1e-8)
            iv = [pers.tile([128, 1], f32, tag=f"iv{fh}", name=f"iv{fh}")
                  for fh in range(FHN)]

            def emit_sq(fh):
                # split per ky so each piece depends on one weight DMA
                fsl = slice(fh * 128, (fh + 1) * 128)
                for ky in (0, 2, 1):
                    for ch in range(CH):
                        nc.scalar.activation(
                            sq[ch][:, 3 * ky:3 * ky + 3, fsl],
                            wraw[ch][:, 3 * ky:3 * ky + 3, fsl],
                            AF.Square, scale=s_t[:, ch:ch + 1])

            DEMOD_TAPS = (0, 1, 2, 6, 7, 8, 3, 4, 5)  # ky1 taps last

            def emit_demod(fh):
                d2 = psd.tile([128, 1], f32, tag="d2", name=f"d2_{fh}")
                i = 0
                for t in DEMOD_TAPS:
                    for ch in range(CH):
                        nc.tensor.matmul(
                            d2[:], sq[ch][:, t, fh * 128:(fh + 1) * 128],
                            ones_t[:], start=(i == 0),
                            stop=(i == CH * NTAP - 1))
                        i += 1
                dm = dtmp.tile([128, 1], f32, tag="dmd", name="dmd")
                nc.scalar.activation(dm[:], d2[:], AF.Sqrt, bias=eps_t[:])
                nc.vector.reciprocal(iv[fh][:], dm[:])

            # ---- startup choreography ----
            dma_wk(0, 0)
            emit_slot_dma(0)
            emit_slot_dma(1)
            # DVE order = consumption order; ch0 pieces first so the
            # very first matmul (j0, ch0) is gated by minimal work
            transform_piece(0, 0, 0, 0, 2)     # chunk0 ht 0-1, ch0
            combos_j0(0)
            transform_piece(0, 1, 0, 0, 2)     # chunk0 ht 0-1, ch1
            dma_wk(2, 0)
            transform_piece(0, 0, 4, 2, 2)     # chunk0 ht 2-3
            transform_piece(0, 1, 4, 2, 2)
            done_chunks.add(0)
            emit_slot_dma(3)
            combos_j3(0)
            dma_wk(1, 0)
            emit_transform(1)
            emit_slot_dma(2)
            combos_j12(0)
            emit_sq(0)

            store_q = [nc.gpsimd, nc.sync, nc.scalar]
            pair_i = 0
            JORD = (0, 3, 1, 2)

            for fh in range(FHN):
                off = 0
                for g, npx in enumerate(GRP):
                    if fh == 0:
                        need = min(NSLOT - 1, (off + npx + 1024) // CLEN)
                        for c in range(need + 1):
                            emit_transform(c)

                    mt = {}
                    for j in JORD:
                        m = ps.tile([128, 512], f32, tag="m",
                                    name=f"m{j}_{g}_{fh}")
                        mt[j] = m
                        i = 0
                        for ch in range(CH):
                            for kx in range(3):
                                rhs = tp[j][ch][:, GT + off + kx - 1:
                                                GT + off + kx - 1 + npx]
                                nc.tensor.matmul(
                                    m[:, :npx],
                                    wt[j][ch][:, kx, fh * 128:(fh + 1) * 128],
                                    rhs, start=(i == 0), stop=(i == 5))
                                i += 1

                    if g == 0:
                        # demod for this f-half: PE stays busy, invd is
                        # ready exactly when group 0's drain needs it
                        emit_demod(fh)
                        if fh == 0:
                            # fh1 prep: emitted early so the (idle-by-
                            # then) queues prefetch; consumed mid-run
                            for ky in (0, 2, 1):
                                dma_wk(ky, 1)
                            combos_j0(1)
                            combos_j3(1)
                            combos_j12(1)
                            emit_sq(1)

                    # ---- drain: ACT scales M_j by invd into bf16,
                    # DVE combines at 2x ----
                    cs = []
                    for j in range(4):
                        cj = dtmp.tile([128, 512], bf16, tag=f"c{j}",
                                       name=f"c{j}")
                        nc.scalar.activation(cj[:, :npx], mt[j][:, :npx],
                                             AF.Copy, scale=iv[fh][:])
                        cs.append(cj)
                    te = dtmp.tile([128, 512], bf16, tag="te", name="te")
                    nc.vector.tensor_add(te[:, :npx], cs[0][:, :npx],
                                         cs[1][:, :npx])
                    to = dtmp.tile([128, 512], bf16, tag="to", name="to")
                    nc.vector.tensor_sub(to[:, :npx], cs[1][:, :npx],
                                         cs[2][:, :npx])
                    ob = stage.tile([128, 2, 512], bf16, tag="ob", name="ob")
                    nc.vector.tensor_add(ob[:, 0, :npx], te[:, :npx],
                                         cs[2][:, :npx])
                    nc.vector.tensor_sub(ob[:, 1, :npx], to[:, :npx],
                                         cs[3][:, :npx])
                    store_q[pair_i % 3].dma_start(
                        yt_d[fh][:, :, off:off + npx], ob[:, :, :npx])
                    pair_i += 1
                    off += npx

    nc.compile()
    return nc


def _get_nc():
    if "nc" not in _COMPILED:
        _COMPILED["nc"] = _build_nc()
    return _COMPILED["nc"]


def _prep_in_maps(x, style, kernel):
    """Host-side layout marshalling: shard over B, transpose+pad+cast x."""
    bf = ml_dtypes.bfloat16
    x = np.ascontiguousarray(x, dtype=np.float32)
    style = np.ascontiguousarray(style, dtype=np.float32)
    kernel = np.ascontiguousarray(kernel, dtype=np.float32)
    # [3,3,C,F] -> [c_half, c_low, tap, f], bf16
    wk = np.ascontiguousarray(
        kernel.reshape(NTAP, CH, 128, F).transpose(1, 2, 0, 3)).astype(bf)
    in_maps = []
    for b in range(B):
        xp = np.zeros((C, HP, WW), dtype=np.float32)
        xp[:, 1:H + 1, 1:W + 1] = x[b].transpose(2, 0, 1)
        xt = np.ascontiguousarray(
            xp.reshape(CH, 128, HP * WW)).astype(bf)
        st = np.ascontiguousarray(style[b].reshape(CH, 128).T)
        in_maps.append({"xt": xt, "st": st, "wk": wk})
    return in_maps


def run_cores(x, style, kernel, trace=False, trace_cores=None):
    """Compile (cached) + run on the 8 NeuronCores. Returns (y, results)."""
    from concourse.bass_utils import run_bass_kernel_spmd

    nc = _get_nc()
    in_maps = _prep_in_maps(x, style, kernel)
    kwargs = {}
    if trace:
        kwargs.update(trace=True, trace_cores=trace_cores)
    res = run_bass_kernel_spmd(nc, in_maps, list(range(N_CORES)), **kwargs)
    y = np.empty((B, H, W, F), dtype=np.float32)
    for b in range(B):
        yt = np.asarray(res.results[b]["yt"]).astype(np.float32)
        # [FHN, 128, 2, TLEN] -> strip pad cols, interleave parity rows
        for p in range(2):
            for fh in range(FHN):
                pl = yt[fh, :, p].reshape(128, NHT, WW)[:, :, 1:W + 1]
                y[b, p::2, :, fh * 128:(fh + 1) * 128] = pl.transpose(1, 2, 0)
    return y, res


def kernel(x, style, kernel):
    y, _ = run_cores(x, style, kernel)
    return y.astype(np.float32)
